# revision 6
# baseline (speedup 1.0000x reference)
"""Trainium2 Bass kernel for nn_CPCircuitLayer (embedding_lookup).

Math: out[b, n] = dot(A[b, idx_s[n]], Bm[b, idx_h[n]]) with
      A = X @ W_seq^T  [S, R],  Bm = X^T @ W_hid^T  [H, R].
Key identity: out[b, n] = M_b[idx_s[n], idx_h[n]] where
      M_b = A @ Bm^T  [S, H]  (cheap rank-R matmul per batch).
So each output needs ONE f32 gathered from a 4 MB table instead of two
R=32 rows (64x less gather traffic than the direct formulation).

Sharding (8 cores, no collectives): core c = (b = c//4, q = c%4) owns
rows [256q, 256q+256) of M_b. Host buckets the N = S*H outputs by
s >> 5 into 32 groups per batch (8 GPSIMD groups x 4 cores); each
group's outputs become one ap_gather index stream (CAP entries, padded
with 0).

Per-core device pipeline:
  1. Load X[b] (bf16), transposed quarter Xq^T, W factor rows, streams.
  2. PE: A_q^T = Wseq Xq^T [32, 256]; Bm^T = Whid X [32, 1024] (psum,
     copied to bf16); M_q = A_q^T^T Bm^T -> 4 psum tiles [128, 512].
  3. DMA M_q psum -> DRAM bounce [256, 1024] f32, then one strided
     broadcast load into table partitions 0::4: partition 16g+4j holds
     rows (32g+8j)..(32g+8j+8) flattened [8192] f32 (local idx =
     (s&7)<<10 | h). Other partitions keep memset junk (ignored).
  4. 4 x ap_gather rounds (num_idxs=CAP/4): group g's stream gathers on
     all 16 of its channels; only channels 0::4 carry the right subset.
  5. DMA out rows 4g+j = partition 16g+4j -> out [32, CAP] f32; host
     un-permutes (res.ravel()[ (4g+j)*CAP + pos ]).
"""

import numpy as np
import ml_dtypes
from contextlib import ExitStack

import concourse.bass as bass
import concourse.mybir as mybir
import concourse.tile as tile
from concourse import bacc

B, S, H, R = 2, 1024, 1024, 32
N = S * H
NCORES = 8
SQ = S // 4           # rows of M per core
RNDS = 9
RND = 4096            # indices per ap_gather round (proven ucode shape)
CAP = RNDS * RND      # 36864 stream capacity per group (mean 32768)
NEL = 8 * H           # table elements per channel (8 rows)

F32 = mybir.dt.float32
BF16 = mybir.dt.bfloat16
I16 = mybir.dt.int16


def _build(reps: int = 1):
    nc = bacc.Bacc()
    x = nc.declare_dram_parameter("x", [S, H], BF16, False)
    xq = nc.declare_dram_parameter("xq", [SQ, H], BF16, False)
    wseq_t = nc.declare_dram_parameter("wseq_t", [H, R], BF16, False)
    whid_t = nc.declare_dram_parameter("whid_t", [S, R], BF16, False)
    idx = nc.declare_dram_parameter("idx", [128, CAP // 16], I16, False)
    out = nc.declare_dram_parameter("out", [32, CAP], F32, True)
    m_dram = nc.dram_tensor("mq", [SQ, H], F32)

    with tile.TileContext(nc) as tc, ExitStack() as ctx:
        base = ctx.enter_context(tc.tile_pool(name="base", bufs=1))
        ppa = ctx.enter_context(tc.tile_pool(name="ppa", bufs=1, space="PSUM"))
        ppb = ctx.enter_context(tc.tile_pool(name="ppb", bufs=2, space="PSUM"))
        ppm = ctx.enter_context(tc.tile_pool(name="ppm", bufs=4, space="PSUM"))
        stage = ctx.enter_context(tc.tile_pool(name="stage", bufs=1))
        gap = ctx.enter_context(tc.tile_pool(name="gap", bufs=2))

        x_sb = base.tile([128, 8, H], BF16)      # X[s,h]: p=s%128, k=s//128
        xqt_sb = base.tile([128, 8, SQ], BF16)   # Xq^T[h,s']: p=h%128
        ws_sb = base.tile([128, 8, R], BF16)     # W_seq^T rows (h-major)
        wh_sb = base.tile([128, 8, R], BF16)     # W_hid^T rows (s-major)
        idx_sb = base.tile([128, CAP // 16], I16)
        table = base.tile([128, NEL], F32)

        nc.sync.dma_start(
            out=x_sb[:],
            in_=bass.AP(tensor=x[:].tensor, offset=0,
                        ap=[[H, 128], [128 * H, 8], [1, H]]),
        )
        for k in range(8):
            nc.sync.dma_start_transpose(
                out=xqt_sb[:, k, :], in_=xq[:, 128 * k:128 * (k + 1)]
            )
        nc.sync.dma_start(
            out=ws_sb[:],
            in_=bass.AP(tensor=wseq_t[:].tensor, offset=0,
                        ap=[[R, 128], [128 * R, 8], [1, R]]),
        )
        nc.sync.dma_start(
            out=wh_sb[:],
            in_=bass.AP(tensor=whid_t[:].tensor, offset=0,
                        ap=[[R, 128], [128 * R, 8], [1, R]]),
        )
        nc.sync.dma_start(out=idx_sb[:], in_=idx[:])
        # partitions not covered by the table load read as junk in the
        # gathers (their output is discarded); memset once so CoreSim sees
        # initialized data.
        nc.vector.memset(table[:], 0.0)

        for _ in range(reps):
            _body(nc, ppa, ppb, ppm, stage, gap,
                  x_sb, xqt_sb, ws_sb, wh_sb, idx_sb, table, m_dram, out)
    nc.compile()
    return nc


def _body(nc, ppa, ppb, ppm, stage, gap,
          x_sb, xqt_sb, ws_sb, wh_sb, idx_sb, table, m_dram, out):
    # --- factors (PE, bf16 in / f32 psum) -------------------------------
    # A_q^T[r, s'] = sum_h Wseq^T[h,r] Xq^T[h,s']
    pa = ppa.tile([R, SQ], F32, tag="pa")
    for k in range(8):
        nc.tensor.matmul(out=pa[:], lhsT=ws_sb[:, k, :], rhs=xqt_sb[:, k, :],
                         start=(k == 0), stop=(k == 7))
    aq = stage.tile([R, SQ], BF16, tag="aq")
    nc.scalar.copy(out=aq[:], in_=pa[:])
    # Bm^T[r, h] = sum_s Whid^T[s,r] X[s,h]
    bm = stage.tile([R, H], BF16, tag="bm")
    for hc in range(2):
        pb = ppb.tile([R, 512], F32, tag="pb")
        for k in range(8):
            nc.tensor.matmul(out=pb[:], lhsT=wh_sb[:, k, :],
                             rhs=x_sb[:, k, 512 * hc:512 * (hc + 1)],
                             start=(k == 0), stop=(k == 7))
        nc.vector.tensor_copy(out=bm[:, 512 * hc:512 * (hc + 1)], in_=pb[:])

    # --- M quarter: M_q[s', h] = sum_r A_q^T[r, s'] Bm^T[r, h] ----------
    m_sb = stage.tile([128, 2, H], F32, tag="msb")
    for rc in range(2):
        for hc in range(2):
            pm = ppm.tile([128, 512], F32, tag="pm")
            nc.tensor.matmul(out=pm[:],
                             lhsT=aq[:, 128 * rc:128 * (rc + 1)],
                             rhs=bm[:, 512 * hc:512 * (hc + 1)],
                             start=True, stop=True)
            eng = nc.vector.tensor_copy if hc == 0 else nc.scalar.copy
            eng(out=m_sb[:, rc, 512 * hc:512 * (hc + 1)], in_=pm[:])
        nc.sync.dma_start(
            out=bass.AP(tensor=m_dram[:].tensor, offset=rc * 128 * H,
                        ap=[[H, 128], [1, H]]),
            in_=m_sb[:, rc, :],
        )

    # --- table: partition 16g+4j <- rows [8*(4g+j), +8) flattened -------
    nc.sync.dma_start(
        out=table[0::4, :],
        in_=bass.AP(tensor=m_dram[:].tensor, offset=0,
                    ap=[[NEL, 32], [1, NEL]]),
    )

    # --- gather + out ----------------------------------------------------
    for rnd in range(RNDS):
        ga = gap.tile([128, RND], F32, tag="ga")
        nc.gpsimd.ap_gather(
            out_ap=ga[:], in_ap=table[:],
            idxs_ap=idx_sb[:, rnd * (RND // 16):(rnd + 1) * (RND // 16)],
            channels=128, num_elems=NEL, d=1, num_idxs=RND,
        )
        nc.sync.dma_start(
            out=bass.AP(tensor=out[:].tensor, offset=rnd * RND,
                        ap=[[CAP, 32], [1, RND]]),
            in_=ga[0::4, :],
        )


_nc_cache_by_reps = {}


def _get_nc(reps: int = 1):
    nc = _nc_cache_by_reps.get(reps)
    if nc is None:
        nc = _nc_cache_by_reps[reps] = _build(reps)
    return nc


class _Runner:
    """Trace/compile the SPMD executable once; reuse across calls."""

    def __init__(self, nc):
        import jax
        from jax.experimental.shard_map import shard_map
        from jax.sharding import Mesh, PartitionSpec
        import concourse.bass2jax as b2j

        b2j.install_neuronx_cc_hook()
        self.nc = nc
        part_name = (nc.partition_id_tensor.name
                     if nc.partition_id_tensor else None)
        in_names, out_names, out_avals = [], [], []
        zero_outs = []
        for alloc in nc.m.functions[0].allocations:
            if not isinstance(alloc, mybir.MemoryLocationSet):
                continue
            name = alloc.memorylocations[0].name
            if alloc.kind == "ExternalInput":
                if name != part_name:
                    in_names.append(name)
            elif alloc.kind == "ExternalOutput":
                out_names.append(name)
                shape = tuple(alloc.tensor_shape)
                dtype = mybir.dt.np(alloc.dtype)
                out_avals.append(jax.core.ShapedArray(shape, dtype))
                zero_outs.append(np.zeros(shape, dtype))
        self.in_names = list(in_names)
        self.out_names = out_names
        self.zero_outs = zero_outs
        n_params = len(in_names)
        n_outs = len(out_names)
        all_in_names = in_names + out_names
        if part_name is not None:
            all_in_names = all_in_names + [part_name]
        donate = tuple(range(n_params, n_params + n_outs))

        def _body_fn(*args):
            operands = list(args)
            if part_name is not None:
                operands.append(b2j.partition_id_tensor())
            outs = b2j._bass_exec_p.bind(
                *operands,
                out_avals=tuple(out_avals),
                in_names=tuple(all_in_names),
                out_names=tuple(out_names),
                lowering_input_output_aliases=(),
                sim_require_finite=True,
                sim_require_nnan=True,
                nc=nc,
            )
            return tuple(outs)

        import os
        plat = "cpu" if os.environ.get("BASS_KERNEL_SIM") else None
        devices = jax.devices(plat)[:NCORES]
        mesh = Mesh(np.asarray(devices), ("core",))
        self.fn = jax.jit(
            shard_map(
                _body_fn, mesh=mesh,
                in_specs=(PartitionSpec("core"),) * (n_params + n_outs),
                out_specs=(PartitionSpec("core"),) * n_outs,
                check_rep=False,
            ),
            donate_argnums=donate,
            keep_unused=True,
        )

    def __call__(self, in_maps):
        concat_in = [
            np.concatenate([np.asarray(m[name]) for m in in_maps], axis=0)
            for name in self.in_names
        ]
        concat_zeros = [
            np.zeros((NCORES * z.shape[0], *z.shape[1:]), z.dtype)
            for z in self.zero_outs
        ]
        out_arrs = self.fn(*concat_in, *concat_zeros)
        return [
            {
                name: np.asarray(out_arrs[i]).reshape(NCORES, -1)[c]
                for i, name in enumerate(self.out_names)
            }
            for c in range(NCORES)
        ]


_runner_cache = {}


def _get_runner(reps: int = 1):
    r = _runner_cache.get(reps)
    if r is None:
        r = _runner_cache[reps] = _Runner(_get_nc(reps))
    return r


def _plan_streams(all_indices):
    """Bucket outputs by s>>5; build wrapped idx streams per quarter and
    the flat positions for un-permuting results."""
    s = all_indices[:, 0].astype(np.int32)
    h = all_indices[:, 1].astype(np.int32)
    bucket = s >> 5                     # 0..31 = (q << 3) | g
    order = np.argsort(bucket, kind="stable")
    bs = bucket[order]
    cnts = np.bincount(bucket, minlength=32)
    offs = np.zeros(33, np.int64)
    np.cumsum(cnts, out=offs[1:])
    pos = np.arange(N, dtype=np.int64) - offs[bs]
    valid = pos < CAP                   # overflow beyond CAP: dropped
    loc = (((s & 7) << 10) | h).astype(np.int16)[order]
    st = np.zeros((32, CAP), np.int16)
    st[bs[valid], pos[valid]] = loc[valid]
    # row in the [32, CAP] result: 4g + j, with g = bucket & 7, j = (s>>3)&3
    row = 4 * (bs & 7) + ((s[order] >> 3) & 3)
    fl = np.minimum(row * CAP + pos, 32 * CAP - 1)
    streams, flats, npos = [], [], []
    for q in range(4):
        sl = slice(offs[8 * q], offs[8 * (q + 1)])
        wq = st[8 * q:8 * (q + 1)].reshape(8, CAP // 16, 16)
        wq = np.ascontiguousarray(wq.transpose(0, 2, 1)).reshape(128, CAP // 16)
        streams.append(wq)
        flats.append(fl[sl])
        npos.append(order[sl])
    return streams, flats, npos


_LAST_PLAN = [None, None]  # (flats, npos) from the last prepare_in_maps


def prepare_in_maps(hidden_states, W_seq, W_hid, all_indices):
    x_bf = [np.ascontiguousarray(hidden_states[b].astype(ml_dtypes.bfloat16))
            for b in range(B)]
    ws_t = np.ascontiguousarray(W_seq.T.astype(ml_dtypes.bfloat16))
    wh_t = np.ascontiguousarray(W_hid.T.astype(ml_dtypes.bfloat16))
    streams, flats, npos = _plan_streams(np.asarray(all_indices))
    _LAST_PLAN[0], _LAST_PLAN[1] = flats, npos
    in_maps = []
    for c in range(NCORES):
        b, q = c // 4, c % 4
        in_maps.append({
            "x": x_bf[b],
            "xq": np.ascontiguousarray(x_bf[b][SQ * q:SQ * (q + 1)]),
            "wseq_t": ws_t,
            "whid_t": wh_t,
            "idx": streams[q],
        })
    return in_maps


def kernel(hidden_states, W_seq, W_hid, all_indices):
    hidden_states = np.asarray(hidden_states)
    W_seq = np.asarray(W_seq)
    W_hid = np.asarray(W_hid)
    all_indices = np.asarray(all_indices)

    runner = _get_runner()
    in_maps = prepare_in_maps(hidden_states, W_seq, W_hid, all_indices)
    flats, npos = _LAST_PLAN
    results = runner(in_maps)

    out = np.empty((B, N), dtype=np.float32)
    for c in range(NCORES):
        b, q = c // 4, c % 4
        out[b, npos[q]] = results[c]["out"].ravel()[flats[q]]
    return out.reshape(B, S, H)


# revision 10
# speedup vs baseline: 61.8196x; 61.8196x over previous
"""Trainium2 Bass kernel for nn_CPCircuitLayer (embedding_lookup).

Math: A = X @ W_seq^T  [S,R];  Bm = X^T @ W_hid^T  [H,R]
      out[b, n] = dot(A[b, idx_s[n]], Bm[b, idx_h[n]]),  out -> [B, S, H]

Sharding (8 cores, no collectives): core c handles batch b = c//4 and the
quarter q = c%4 of the N = S*H index list (J = N/4 indices). Both factor
tables are computed redundantly per batch group from the full X[b].

Per-core device pipeline:
  1. Load X[b] (bf16) + transposed copy via HWDGE transpose-DMA.
  2. PE matmuls (bf16 in, f32 psum): A^T and Bm^T [32, 1024].
  3. Repack to per-lane split-R tables: partition p holds columns
     2*(p%16), 2*(p%16)+1 of the factor interleaved ([128, 1024, 2] f32),
     via a DRAM bounce + 8x partition-group broadcast load.
  4. ap_gather (GPSIMD FIFO): each 16-partition group streams its own
     indices; one instruction gathers NIdx rows x 8 groups.
  5. DVE mul + pair-sum, then PE block-indicator matmul reduces the 16
     lanes x 2 of each group -> psum [8, n] -> out.
"""

import numpy as np
import ml_dtypes
from contextlib import ExitStack

import concourse.bass as bass
import concourse.mybir as mybir
import concourse.tile as tile
from concourse import bacc

B, S, H, R = 2, 1024, 1024, 32
N = S * H
NCORES = 8
J = N // 4            # outputs per core (one batch, quarter of N) = 262144
JG = J // 8           # outputs per 16-partition group = 32768
NIdx = 2048           # indices per group per ap_gather instruction
RNDS = JG // NIdx     # 16 gather rounds per table
GRP_D = 2             # table f32 per lane (R = 16 lanes * 2)
SKIP_GATHER = False   # timing experiment: drop ap_gather instructions

F32 = mybir.dt.float32
BF16 = mybir.dt.bfloat16
I16 = mybir.dt.int16


def _build(reps: int = 1):
    nc = bacc.Bacc()
    x = nc.declare_dram_parameter("x", [S, H], BF16, False)
    wseq_t = nc.declare_dram_parameter("wseq_t", [H, R], BF16, False)
    whid_t = nc.declare_dram_parameter("whid_t", [S, R], BF16, False)
    # per-group index streams, wrapped: group g's jj-th index lives at
    # [16*g + jj%16, jj//16]
    idx_s = nc.declare_dram_parameter("idx_s", [128, 2 * JG // 16], I16, False)
    idx_h = nc.declare_dram_parameter("idx_h", [128, 2 * JG // 16], I16, False)
    ind_in = nc.declare_dram_parameter("ind", [128, 8], F32, False)
    out = nc.declare_dram_parameter("out", [8, JG], F32, True)
    ta_dram = nc.dram_tensor("ta", [R, S], F32)   # A^T bounce
    tb_dram = nc.dram_tensor("tb", [R, H], F32)   # Bm^T bounce

    with tile.TileContext(nc) as tc, ExitStack() as ctx:
        base = ctx.enter_context(tc.tile_pool(name="base", bufs=1))
        psum = ctx.enter_context(tc.tile_pool(name="psum", bufs=2, space="PSUM"))
        rpsum = ctx.enter_context(tc.tile_pool(name="rpsum", bufs=1, space="PSUM"))
        stage = ctx.enter_context(tc.tile_pool(name="stage", bufs=2))
        gap = ctx.enter_context(tc.tile_pool(name="gap", bufs=2))
        gbp = ctx.enter_context(tc.tile_pool(name="gbp", bufs=2))
        prodp = ctx.enter_context(tc.tile_pool(name="prodp", bufs=2))
        otp = ctx.enter_context(tc.tile_pool(name="otp", bufs=1))

        # --- loads -------------------------------------------------------
        x_sb = base.tile([128, 8, H], BF16)       # X[s,h]: p=s%128, k=s//128
        xt_sb = base.tile([128, 8, S], BF16)      # X^T[h,s]: p=h%128, k=h//128
        ws_sb = base.tile([128, 8, R], BF16)      # W_seq^T rows (h-major)
        wh_sb = base.tile([128, 8, R], BF16)      # W_hid^T rows (s-major)
        isb_s = base.tile([128, 2 * JG // 16], I16)
        isb_h = base.tile([128, 2 * JG // 16], I16)
        ind_sb = base.tile([128, 8], F32)         # block indicator for reduce
        ta_sb = base.tile([128, 2 * S], F32)
        tb_sb = base.tile([128, 2 * H], F32)

        nc.sync.dma_start(
            out=x_sb[:],
            in_=bass.AP(tensor=x[:].tensor, offset=0,
                        ap=[[H, 128], [128 * H, 8], [1, H]]),
        )
        for k in range(8):
            nc.sync.dma_start_transpose(
                out=xt_sb[:, k, :], in_=x[:, 128 * k:128 * (k + 1)]
            )
        nc.sync.dma_start(
            out=ws_sb[:],
            in_=bass.AP(tensor=wseq_t[:].tensor, offset=0,
                        ap=[[R, 128], [128 * R, 8], [1, R]]),
        )
        nc.sync.dma_start(
            out=wh_sb[:],
            in_=bass.AP(tensor=whid_t[:].tensor, offset=0,
                        ap=[[R, 128], [128 * R, 8], [1, R]]),
        )
        nc.sync.dma_start(out=isb_s[:], in_=idx_s[:])
        nc.sync.dma_start(out=isb_h[:], in_=idx_h[:])

        nc.sync.dma_start(out=ind_sb[:], in_=ind_in[:])

        for _ in range(reps):
            _body(nc, psum, rpsum, stage, gap, gbp, prodp, otp,
                  x_sb, xt_sb, ws_sb, wh_sb, isb_s, isb_h, ind_sb,
                  ta_sb, tb_sb, ta_dram, tb_dram, out)
    nc.compile()
    return nc


def _body(nc, psum, rpsum, stage, gap, gbp, prodp, otp,
          x_sb, xt_sb, ws_sb, wh_sb, isb_s, isb_h, ind_sb,
          ta_sb, tb_sb, ta_dram, tb_dram, out):
    # --- factor transposes on PE: F^T [32, 1024] ------------------------
    # A^T[r, s] = sum_h Wseq^T[h, r] X^T[h, s]; Bm^T[r, h] = sum_s ...
    for (tdram, lhs_w, rhs_x) in ((ta_dram, ws_sb, xt_sb),
                                  (tb_dram, wh_sb, x_sb)):
        ft = stage.tile([R, S], F32, tag="ft")
        for nh in range(2):
            pt = psum.tile([R, S // 2], F32, tag="pt")
            for k in range(8):
                nc.tensor.matmul(
                    out=pt[:],
                    lhsT=lhs_w[:, k, :],
                    rhs=rhs_x[:, k, nh * 512:(nh + 1) * 512],
                    start=(k == 0), stop=(k == 7),
                )
            nc.vector.tensor_copy(out=ft[:, nh * 512:(nh + 1) * 512], in_=pt[:])
        nc.gpsimd.dma_start(out=tdram[:], in_=ft[:])

    # broadcast tables back, lane-split d=2 interleaved: partition p
    # (lane l = p%16) holds tab[p, v, d] = F^T[2l+d, v]
    for (tdram, tsb, V) in ((ta_dram, ta_sb, S), (tb_dram, tb_sb, H)):
        nc.gpsimd.dma_start(
            out=tsb[:],
            in_=bass.AP(tensor=tdram[:].tensor, offset=0,
                        ap=[[0, 8], [2 * V, 16], [1, 2 * V]]),
        )

    # --- gather + reduce ------------------------------------------------
    ot = None
    for rnd in range(RNDS):
        isl = slice(rnd * (2 * NIdx // 16), (rnd + 1) * (2 * NIdx // 16))
        ga = gap.tile([128, NIdx, GRP_D], F32, tag="ga")
        gb = gbp.tile([128, NIdx, GRP_D], F32, tag="gb")
        ga_flat = bass.AP(tensor=ga[:].tensor, offset=ga[:].offset,
                          ap=[list(ga[:].ap[0]), [1, 2 * NIdx], [1, 1]])
        gb_flat = bass.AP(tensor=gb[:].tensor, offset=gb[:].offset,
                          ap=[list(gb[:].ap[0]), [1, 2 * NIdx], [1, 1]])
        if SKIP_GATHER:
            nc.vector.memset(ga[:], 0.0)
            nc.vector.memset(gb[:], 0.0)
        else:
            nc.gpsimd.ap_gather(
                out_ap=ga_flat, in_ap=ta_sb[:], idxs_ap=isb_s[:, isl],
                channels=128, num_elems=2 * S, d=1, num_idxs=2 * NIdx,
            )
            nc.gpsimd.ap_gather(
                out_ap=gb_flat, in_ap=tb_sb[:], idxs_ap=isb_h[:, isl],
                channels=128, num_elems=2 * H, d=1, num_idxs=2 * NIdx,
            )
        prod = prodp.tile([128, NIdx, GRP_D], F32, tag="prod")
        nc.vector.tensor_mul(prod[:], ga[:], gb[:])
        p2 = prodp.tile([128, NIdx], F32, tag="p2")
        nc.vector.tensor_add(p2[:], prod[:, :, 0], prod[:, :, 1])
        # reduce 16 lanes per group via block-indicator matmul; all four
        # 512-col results land in one 4-bank psum tile -> single copy;
        # out-DMA once per two rounds
        if rnd % 2 == 0:
            ot = otp.tile([8, 2 * NIdx], F32, tag="ot")
        rp4 = rpsum.tile([8, NIdx], F32, tag="rp4")
        for t in range(NIdx // 512):
            nc.tensor.matmul(
                out=rp4[:, t * 512:(t + 1) * 512],
                lhsT=ind_sb[:],
                rhs=p2[:, t * 512:(t + 1) * 512],
                start=True, stop=True,
            )
        half = (rnd % 2) * NIdx
        nc.scalar.copy(out=ot[:, half:half + NIdx], in_=rp4[:])
        if rnd % 2 == 1:
            nc.sync.dma_start(
                out=bass.AP(tensor=out[:].tensor, offset=(rnd - 1) * NIdx,
                            ap=[[JG, 8], [1, 2 * NIdx]]),
                in_=ot[:],
            )


_nc_cache_by_reps = {}


def _get_nc(reps: int = 1):
    nc = _nc_cache_by_reps.get(reps)
    if nc is None:
        nc = _nc_cache_by_reps[reps] = _build(reps)
    return nc


class _Runner:
    """Trace/compile the SPMD executable once; reuse across calls."""

    def __init__(self, nc):
        import jax
        from jax.experimental.shard_map import shard_map
        from jax.sharding import Mesh, PartitionSpec
        import concourse.bass2jax as b2j

        b2j.install_neuronx_cc_hook()
        self.nc = nc
        part_name = (nc.partition_id_tensor.name
                     if nc.partition_id_tensor else None)
        in_names, out_names, out_avals = [], [], []
        zero_outs = []
        for alloc in nc.m.functions[0].allocations:
            if not isinstance(alloc, mybir.MemoryLocationSet):
                continue
            name = alloc.memorylocations[0].name
            if alloc.kind == "ExternalInput":
                if name != part_name:
                    in_names.append(name)
            elif alloc.kind == "ExternalOutput":
                out_names.append(name)
                shape = tuple(alloc.tensor_shape)
                dtype = mybir.dt.np(alloc.dtype)
                out_avals.append(jax.core.ShapedArray(shape, dtype))
                zero_outs.append(np.zeros(shape, dtype))
        self.in_names = list(in_names)
        self.out_names = out_names
        self.zero_outs = zero_outs
        n_params = len(in_names)
        n_outs = len(out_names)
        all_in_names = in_names + out_names
        if part_name is not None:
            all_in_names = all_in_names + [part_name]
        donate = tuple(range(n_params, n_params + n_outs))

        def _body_fn(*args):
            operands = list(args)
            if part_name is not None:
                operands.append(b2j.partition_id_tensor())
            outs = b2j._bass_exec_p.bind(
                *operands,
                out_avals=tuple(out_avals),
                in_names=tuple(all_in_names),
                out_names=tuple(out_names),
                lowering_input_output_aliases=(),
                sim_require_finite=True,
                sim_require_nnan=True,
                nc=nc,
            )
            return tuple(outs)

        devices = jax.devices()[:NCORES]
        mesh = Mesh(np.asarray(devices), ("core",))
        self.fn = jax.jit(
            shard_map(
                _body_fn, mesh=mesh,
                in_specs=(PartitionSpec("core"),) * (n_params + n_outs),
                out_specs=(PartitionSpec("core"),) * n_outs,
                check_rep=False,
            ),
            donate_argnums=donate,
            keep_unused=True,
        )

    def __call__(self, in_maps):
        concat_in = [
            np.concatenate([np.asarray(m[name]) for m in in_maps], axis=0)
            for name in self.in_names
        ]
        concat_zeros = [
            np.zeros((NCORES * z.shape[0], *z.shape[1:]), z.dtype)
            for z in self.zero_outs
        ]
        out_arrs = self.fn(*concat_in, *concat_zeros)
        return [
            {
                name: np.asarray(out_arrs[i]).reshape(NCORES, -1)[c]
                for i, name in enumerate(self.out_names)
            }
            for c in range(NCORES)
        ]


_runner_cache = {}


def _get_runner(reps: int = 1):
    r = _runner_cache.get(reps)
    if r is None:
        r = _runner_cache[reps] = _Runner(_get_nc(reps))
    return r


def _wrap_idx(v: np.ndarray) -> np.ndarray:
    """[J] -> [128, 2*JG/16] int16: group g = j // JG streams the pairs
    (v, v+1024) for its outputs, wrapped at [16*g + t%16, t//16]."""
    v = v.astype(np.int16)
    v2 = np.empty(2 * J, np.int16)
    v2[0::2] = v
    v2[1::2] = v + 1024
    w = v2.reshape(8, 2 * JG // 16, 16)   # [g, col, p16]
    w = w.transpose(0, 2, 1).reshape(128, 2 * JG // 16)
    return np.ascontiguousarray(w)


def prepare_in_maps(hidden_states, W_seq, W_hid, all_indices):
    x_bf = [np.ascontiguousarray(hidden_states[b].astype(ml_dtypes.bfloat16))
            for b in range(B)]
    ws_t = np.ascontiguousarray(W_seq.T.astype(ml_dtypes.bfloat16))
    wh_t = np.ascontiguousarray(W_hid.T.astype(ml_dtypes.bfloat16))
    idx_pairs = []
    for q in range(4):
        seg = all_indices[q * J:(q + 1) * J]
        idx_pairs.append((_wrap_idx(seg[:, 0]), _wrap_idx(seg[:, 1])))
    in_maps = []
    for c in range(NCORES):
        b, q = c // 4, c % 4
        ind = np.zeros((128, 8), np.float32)
        for g in range(8):
            ind[16 * g:16 * (g + 1), g] = 1.0
        in_maps.append({
            "x": x_bf[b],
            "wseq_t": ws_t,
            "whid_t": wh_t,
            "idx_s": idx_pairs[q][0],
            "idx_h": idx_pairs[q][1],
            "ind": ind,
        })
    return in_maps


def kernel(hidden_states, W_seq, W_hid, all_indices):
    hidden_states = np.asarray(hidden_states)
    W_seq = np.asarray(W_seq)
    W_hid = np.asarray(W_hid)
    all_indices = np.asarray(all_indices)

    runner = _get_runner()
    in_maps = prepare_in_maps(hidden_states, W_seq, W_hid, all_indices)
    results = runner(in_maps)

    out = np.empty((B, N), dtype=np.float32)
    for c in range(NCORES):
        b, q = c // 4, c % 4
        o = results[c]["out"].reshape(8, JG)
        # out[g, jj] holds output j = g*JG + jj of this core's quarter
        out[b, q * J:(q + 1) * J] = o.reshape(J)
    return out.reshape(B, S, H)

